# revision 9
# baseline (speedup 1.0000x reference)
"""Trainium2 Bass kernel for nn_Aggregation (sparse local attention aggregation).

out[n, g*64+cw, y, x] = sum_{i,j in 3x3} input[n, g*64+cw, y+i-1, x+j-1]
                        * weight[n, cw, i*3+j, y*64+x]

Sharding: data-parallel over batch n: 8 cores x 2 batches each.

Per-core layouts (host pre-swizzled, fp16 on the wire so HBM traffic is
halved; every DMA is a 2-dim [128 partitions x contiguous] transfer):
  x_t : [128=(b,cw), H, (g, 66)]  column-padded: [0, x0..x63, 0] per group,
        so the DMA band load lands directly as the zero-padded "even"
        shift plane (no on-chip scatter).
  w_t : [128=(b,cw), H, (ij, x)]
  o_t : [128=(b,cw), H, (g, x)]   fp16, host upcasts to f32.

Engine split per band (8 output rows), from trace analysis: the PE
identity-matmul accumulate costs ~0.77 ns/col (512-col matmul cap plus
~173 ns serial SBUF-access latency per matmul), DVE tensor ops run 2x
mode at ~0.54 ns/elem, Pool (gpsimd) multiplies at ~2 ns/elem. Balance:
  DMA : x/w band loads, out store (fp16).
  ACT : xe->xo shifted copy (odd-parity plane for aligned dj=0 reads),
        PSUM->SBUF evacuation with f32->fp16 cast.
  DVE : 7 of 9 shifted products + 2 tree-adds merging 3 planes into one
        partial sum.
  Pool: 2 of 9 products + edge-band halo memsets.
  PE  : 7 identity-matmul accumulate passes (6 raw planes + the DVE
        partial) into PSUM, fp16 moving data.
"""

import numpy as np

N, C, H, W = 16, 512, 64, 64
CW, G, K = 64, 8, 3
NCORE = 8
NB = N // NCORE          # batches per core
L = H * W

R = 8                    # band rows (output rows per band)
RP = R + 2               # plane rows incl. halo
NBANDS = H // R
WP = W + 2               # 66
GWP = G * WP             # one padded row-block (all groups)
WROW = K * K * W         # 576

POOL_IJ = (0, 2)         # product planes computed on Pool (gpsimd)
TREE_IJ = (0, 2, 8)      # planes merged by DVE adds into one partial
# PE raw passes = the remaining 6 planes

_cache = {}


def _build():
    import concourse.mybir as mybir
    from concourse import bacc
    from concourse.tile import TileContext
    from concourse.masks import make_identity

    f16 = mybir.dt.float16
    f32 = mybir.dt.float32

    nc = bacc.Bacc()
    x_t = nc.dram_tensor("x_t", [128, H, GWP], f16, kind="ExternalInput")
    w_t = nc.dram_tensor("w_t", [128, H, WROW], f16, kind="ExternalInput")
    o_t = nc.dram_tensor("o_t", [128, H, G * W], f16, kind="ExternalOutput")

    PL = RP * GWP            # padded plane length per partition

    with TileContext(nc) as tc:
        with (
            tc.tile_pool(name="const", bufs=1) as const_pool,
            tc.tile_pool(name="xe", bufs=2) as xe_pool,
            tc.tile_pool(name="wt", bufs=2) as wt_pool,
            tc.tile_pool(name="pr", bufs=2) as pr_pool,
            tc.tile_pool(name="os", bufs=2) as os_pool,
            tc.tile_pool(name="ps", bufs=2, space="PSUM") as ps_pool,
        ):
            # Two identity copies: alternating the stationary operand lets
            # each LDWEIGHTS target the background weight buffer and overlap
            # the in-flight matmul (same-tensor LDW serializes instead).
            ident = const_pool.tile([128, 128], f16)
            make_identity(nc, ident)
            ident2 = const_pool.tile([128, 128], f16)
            make_identity(nc, ident2)
            idents = [ident, ident2]

            for band in range(NBANDS):
                y0 = band * R
                row_lo = max(y0 - 1, 0)             # first loaded image row
                row_hi = min(y0 + R + 1, H)         # one past last loaded row
                RL = row_hi - row_lo                # rows loaded
                prow0 = 0 if y0 > 0 else 1          # plane row of first loaded row

                # ---- load x band straight into the padded even plane
                # (+66 slack: shifted APs over-run the last plane row)
                xe = xe_pool.tile([128, PL + 66], f16, tag="xe")
                nc.sync.dma_start(
                    out=xe[:, prow0 * GWP : (prow0 + RL) * GWP],
                    in_=x_t[:, row_lo:row_hi, :],
                )
                if band == 0:
                    nc.gpsimd.memset(xe[:, 0:GWP], 0.0)
                if band == NBANDS - 1:
                    nc.gpsimd.memset(xe[:, (RP - 1) * GWP : RP * GWP], 0.0)

                # ---- odd-parity plane: xo[., g, c] = x[c]  (= xe col c+1)
                xo = xe_pool.tile([128, PL + 66], f16, tag="xo")
                src = xe[:, :PL].rearrange("p (r g c) -> p r g c", g=G, c=WP)[
                    :, :, :, 1 : W + 1
                ]
                dst = xo[:, :PL].rearrange("p (r g c) -> p r g c", g=G, c=WP)[
                    :, :, :, 0:W
                ]
                nc.scalar.copy(out=dst, in_=src)

                # ---- load weight band
                wt = wt_pool.tile([128, R * WROW], f16, tag="wt")
                nc.sync.dma_start(out=wt[:], in_=w_t[:, y0 : y0 + R, :])
                wv = wt[:].rearrange("p (r s) -> p r s", s=WROW)

                # ---- 9 shifted products (7 DVE, 2 Pool). Pool planes are
                # slow but start right after the xe DMA (no xo dependency)
                # and are consumed by the LAST PE passes, so their latency
                # hides under the DVE planes.
                planes = [None] * (K * K)
                dve_tag = 0
                for ij in range(K * K):
                    di, dj = ij // K - 1, ij % K - 1
                    if dj == 0:
                        plane, col0 = xo, 0
                    else:
                        plane, col0 = xe, 1 + dj    # dj=-1 -> 0, dj=+1 -> 2
                    off = (1 + di) * GWP + col0
                    xsrc = (
                        plane[:, off : off + R * GWP]
                        .rearrange("p (r gc) -> p r gc", gc=GWP)
                        .rearrange("p r (g c) -> p r g c", c=WP)[:, :, :, 0:W]
                    )
                    wsrc = (
                        wv[:, :, ij * W : (ij + 1) * W]
                        .unsqueeze(2)
                        .broadcast_to([128, R, G, W])
                    )
                    if ij in POOL_IJ:
                        tag = f"prP{POOL_IJ.index(ij)}"
                        eng = nc.gpsimd
                    else:
                        tag = f"pr{dve_tag % 4}"
                        dve_tag += 1
                        eng = nc.vector
                    pr = pr_pool.tile([128, R * G * W], f16, tag=tag)
                    prv = pr.rearrange("p (r g c) -> p r g c", g=G, c=W)
                    eng.tensor_mul(out=prv, in0=xsrc, in1=wsrc)
                    planes[ij] = pr

                # ---- PE accumulate: all 9 planes, 8 x 512-col matmuls per
                # pass across two 4-bank PSUM tiles; DVE planes first, Pool
                # planes last
                passes = [planes[ij] for ij in range(K * K) if ij not in POOL_IJ]
                passes += [planes[ij] for ij in POOL_IJ]
                os_ = os_pool.tile([128, R * G * W], f16, tag="os")
                mm = 0
                for half in range(2):
                    ps_tiles = [
                        ps_pool.tile(
                            [128, 2 * 512], f32, tag=f"ps{t}",
                            name=f"ps_{band}_{half}_{t}",
                        )
                        for t in range(2)
                    ]
                    for pi, pr in enumerate(passes):
                        for c in range(4):
                            cc = half * 4 + c
                            nc.tensor.matmul(
                                ps_tiles[c // 2][:, (c % 2) * 512 : (c % 2 + 1) * 512],
                                idents[mm % 2],
                                pr[:, cc * 512 : (cc + 1) * 512],
                                start=(pi == 0),
                                stop=(pi == len(passes) - 1),
                            )
                            mm += 1
                    # evacuate this half's PSUM (f32 -> fp16)
                    for t in range(2):
                        nc.scalar.copy(
                            out=os_[
                                :,
                                half * 2048 + t * 1024 : half * 2048 + (t + 1) * 1024,
                            ],
                            in_=ps_tiles[t],
                        )
                nc.sync.dma_start(out=o_t[:, y0 : y0 + R, :], in_=os_[:])

    nc.finalize()
    return nc


def _get():
    if "nc" not in _cache:
        _cache["nc"] = _build()
    return _cache["nc"]


def _swizzle_core(inp, wgt):
    # inp [2, 512, 64, 64] -> [128, H, G*66] fp16, zero-padded columns;
    # p = b*64+cw, free = (y, g, 66)
    a = inp.reshape(NB, G, CW, H, W).transpose(0, 2, 3, 1, 4)  # b,cw,y,g,x
    xe = np.zeros((NB, CW, H, G, WP), dtype=np.float16)
    xe[..., 1 : W + 1] = a
    xe = xe.reshape(128, H, GWP)
    # wgt [2, 64, 9, 4096] -> [128, H, 9*W] fp16; free = (y, ij, x)
    b = wgt.reshape(NB, CW, K * K, H, W).transpose(0, 1, 3, 2, 4)
    wt = np.ascontiguousarray(b, dtype=np.float16).reshape(128, H, WROW)
    return xe, wt


def _unswizzle_core(o):
    # [128, H, G*W] fp16 -> [2, 512, 64, 64] f32
    a = o.reshape(NB, CW, H, G, W).astype(np.float32).transpose(0, 3, 1, 2, 4)
    return np.ascontiguousarray(a).reshape(NB, C, H, W)


def kernel(input: np.ndarray, weight: np.ndarray) -> np.ndarray:
    from concourse.bass_utils import run_bass_kernel_spmd

    input = np.ascontiguousarray(input, dtype=np.float32)
    weight = np.ascontiguousarray(weight, dtype=np.float32)
    nc = _get()
    in_maps = []
    for i in range(NCORE):
        a, b = _swizzle_core(
            input[i * NB : (i + 1) * NB], weight[i * NB : (i + 1) * NB]
        )
        in_maps.append({"x_t": a, "w_t": b})
    res = run_bass_kernel_spmd(nc, in_maps, core_ids=list(range(NCORE)))
    return np.concatenate(
        [_unswizzle_core(res.results[i]["o_t"]) for i in range(NCORE)], axis=0
    )


# revision 13
# speedup vs baseline: 1.4753x; 1.4753x over previous
"""Trainium2 Bass kernel for nn_Aggregation (sparse local attention aggregation).

out[n, g*64+cw, y, x] = sum_{i,j in 3x3} input[n, g*64+cw, y+i-1, x+j-1]
                        * weight[n, cw, i*3+j, y*64+x]

Sharding: data-parallel over batch n: 8 cores x 2 batches each.

Per-core layouts (host pre-swizzled, fp16 on the wire so HBM traffic is
halved; every DMA is a 2-dim [128 partitions x contiguous] transfer):
  x_t : [128=(b,cw), H, (g, 66)]  column-padded: [0, x0..x63, 0] per group,
        so the DMA band load lands directly as the zero-padded "even"
        shift plane (no on-chip scatter).
  w_t : [128=(b,cw), H, (ij, x)]
  o_t : [128=(b,cw), H, (g, x)]   fp16, host upcasts to f32.

Engine split per band (8 output rows), from trace analysis: the PE
identity-matmul accumulate costs ~0.77 ns/col (512-col matmul cap plus
~173 ns serial SBUF-access latency per matmul), DVE tensor ops run 2x
mode at ~0.54 ns/elem, Pool (gpsimd) multiplies at ~2 ns/elem. Balance:
  DMA : x/w band loads, out store (fp16).
  ACT : xe->xo shifted copy (odd-parity plane for aligned dj=0 reads),
        PSUM->SBUF evacuation with f32->fp16 cast.
  DVE : 7 of 9 shifted products + 2 tree-adds merging 3 planes into one
        partial sum.
  Pool: 2 of 9 products + edge-band halo memsets.
  PE  : 7 identity-matmul accumulate passes (6 raw planes + the DVE
        partial) into PSUM, fp16 moving data.
"""

import numpy as np

N, C, H, W = 16, 512, 64, 64
CW, G, K = 64, 8, 3
NCORE = 8
NB = N // NCORE          # batches per core
L = H * W

R = 8                    # band rows (output rows per band)
RP = R + 2               # plane rows incl. halo
NBANDS = H // R
WP = W + 2               # 66
GWP = G * WP             # one padded row-block (all groups)
WROW = K * K * W         # 576

POOL_IJ = ()             # Pool (gpsimd) tensor ops measured 4-15x slower than
                         # DVE on these strided/broadcast APs (software
                         # addressing on Q7) — products stay on DVE

_cache = {}


def _build():
    import concourse.mybir as mybir
    from concourse import bacc
    from concourse.tile import TileContext
    from concourse.masks import make_identity

    f16 = mybir.dt.float16
    f32 = mybir.dt.float32

    nc = bacc.Bacc()
    x_t = nc.dram_tensor("x_t", [128, H, GWP], f16, kind="ExternalInput")
    w_t = nc.dram_tensor("w_t", [128, H, WROW], f16, kind="ExternalInput")
    o_t = nc.dram_tensor("o_t", [128, H, G * W], f16, kind="ExternalOutput")

    PL = RP * GWP            # padded plane length per partition

    with TileContext(nc) as tc:
        with (
            tc.tile_pool(name="const", bufs=1) as const_pool,
            tc.tile_pool(name="xe", bufs=3) as xe_pool,
            tc.tile_pool(name="wt", bufs=3) as wt_pool,
            tc.tile_pool(name="pr", bufs=2) as pr_pool,
            tc.tile_pool(name="os", bufs=2) as os_pool,
            tc.tile_pool(name="ps", bufs=1, space="PSUM") as ps_pool,
        ):
            # Two identity copies: alternating the stationary operand lets
            # each LDWEIGHTS target the background weight buffer and overlap
            # the in-flight matmul (same-tensor LDW serializes instead).
            ident = const_pool.tile([128, 128], f16)
            make_identity(nc, ident)
            ident2 = const_pool.tile([128, 128], f16)
            make_identity(nc, ident2)
            idents = [ident, ident2]

            for band in range(NBANDS):
                y0 = band * R
                row_lo = max(y0 - 1, 0)             # first loaded image row
                row_hi = min(y0 + R + 1, H)         # one past last loaded row
                RL = row_hi - row_lo                # rows loaded
                prow0 = 0 if y0 > 0 else 1          # plane row of first loaded row

                # ---- load x band straight into the padded even plane
                # (+66 slack: shifted APs over-run the last plane row)
                xe = xe_pool.tile([128, PL + 66], f16, tag="xe")
                nc.sync.dma_start(
                    out=xe[:, prow0 * GWP : (prow0 + RL) * GWP],
                    in_=x_t[:, row_lo:row_hi, :],
                )
                if band == 0:
                    nc.gpsimd.memset(xe[:, 0:GWP], 0.0)
                if band == NBANDS - 1:
                    nc.gpsimd.memset(xe[:, (RP - 1) * GWP : RP * GWP], 0.0)

                # ---- odd-parity plane: xo[., g, c] = x[c]  (= xe col c+1)
                xo = xe_pool.tile([128, PL + 66], f16, tag="xo")
                src = xe[:, :PL].rearrange("p (r g c) -> p r g c", g=G, c=WP)[
                    :, :, :, 1 : W + 1
                ]
                dst = xo[:, :PL].rearrange("p (r g c) -> p r g c", g=G, c=WP)[
                    :, :, :, 0:W
                ]
                nc.scalar.copy(out=dst, in_=src)

                # ---- load weight band
                wt = wt_pool.tile([128, R * WROW], f16, tag="wt")
                nc.sync.dma_start(out=wt[:], in_=w_t[:, y0 : y0 + R, :])
                wv = wt[:].rearrange("p (r s) -> p r s", s=WROW)

                # ---- 9 shifted products (7 DVE, 2 Pool). Pool planes are
                # slow but start right after the xe DMA (no xo dependency)
                # and are consumed by the LAST PE passes, so their latency
                # hides under the DVE planes.
                planes = [None] * (K * K)
                dve_tag = 0
                for ij in range(K * K):
                    di, dj = ij // K - 1, ij % K - 1
                    if dj == 0:
                        plane, col0 = xo, 0
                    else:
                        plane, col0 = xe, 1 + dj    # dj=-1 -> 0, dj=+1 -> 2
                    off = (1 + di) * GWP + col0
                    xsrc = (
                        plane[:, off : off + R * GWP]
                        .rearrange("p (r gc) -> p r gc", gc=GWP)
                        .rearrange("p r (g c) -> p r g c", c=WP)[:, :, :, 0:W]
                    )
                    wsrc = (
                        wv[:, :, ij * W : (ij + 1) * W]
                        .unsqueeze(2)
                        .broadcast_to([128, R, G, W])
                    )
                    if ij in POOL_IJ:
                        tag = f"prP{POOL_IJ.index(ij)}"
                        eng = nc.gpsimd
                    else:
                        tag = f"pr{dve_tag % 4}"
                        dve_tag += 1
                        eng = nc.vector
                    pr = pr_pool.tile([128, R * G * W], f16, tag=tag)
                    prv = pr.rearrange("p (r g c) -> p r g c", g=G, c=W)
                    eng.tensor_mul(out=prv, in0=xsrc, in1=wsrc)
                    planes[ij] = pr

                # ---- PE accumulate: all 9 planes, 8 x 512-col matmuls per
                # pass across two 4-bank PSUM tiles; DVE planes first, Pool
                # planes last
                passes = [planes[ij] for ij in range(K * K) if ij not in POOL_IJ]
                passes += [planes[ij] for ij in POOL_IJ]
                os_ = os_pool.tile([128, R * G * W], f16, tag="os")
                # Full-band accumulation: each pass consumes one whole plane
                # (8 matmuls across two 4-bank PSUM tiles), so product tiles
                # are consumed promptly and rotating-tag reuse cannot cycle.
                ps_tiles = [
                    ps_pool.tile(
                        [128, 4 * 512], f32, tag=f"ps{t}", name=f"ps_{band}_{t}"
                    )
                    for t in range(2)
                ]
                mm = 0
                for pi, pr in enumerate(passes):
                    for c in range(8):
                        nc.tensor.matmul(
                            ps_tiles[c // 4][:, (c % 4) * 512 : (c % 4 + 1) * 512],
                            idents[mm % 2],
                            pr[:, c * 512 : (c + 1) * 512],
                            start=(pi == 0),
                            stop=(pi == len(passes) - 1),
                        )
                        mm += 1
                # evacuate PSUM (f32 -> fp16)
                for t in range(2):
                    nc.scalar.copy(
                        out=os_[:, t * 2048 : (t + 1) * 2048], in_=ps_tiles[t]
                    )
                nc.sync.dma_start(out=o_t[:, y0 : y0 + R, :], in_=os_[:])

    nc.finalize()
    return nc


def _get():
    if "nc" not in _cache:
        _cache["nc"] = _build()
    return _cache["nc"]


def _swizzle_core(inp, wgt):
    # inp [2, 512, 64, 64] -> [128, H, G*66] fp16, zero-padded columns;
    # p = b*64+cw, free = (y, g, 66)
    a = inp.reshape(NB, G, CW, H, W).transpose(0, 2, 3, 1, 4)  # b,cw,y,g,x
    xe = np.zeros((NB, CW, H, G, WP), dtype=np.float16)
    xe[..., 1 : W + 1] = a
    xe = xe.reshape(128, H, GWP)
    # wgt [2, 64, 9, 4096] -> [128, H, 9*W] fp16; free = (y, ij, x)
    b = wgt.reshape(NB, CW, K * K, H, W).transpose(0, 1, 3, 2, 4)
    wt = np.ascontiguousarray(b, dtype=np.float16).reshape(128, H, WROW)
    return xe, wt


def _unswizzle_core(o):
    # [128, H, G*W] fp16 -> [2, 512, 64, 64] f32
    a = o.reshape(NB, CW, H, G, W).astype(np.float32).transpose(0, 3, 1, 2, 4)
    return np.ascontiguousarray(a).reshape(NB, C, H, W)


def kernel(input: np.ndarray, weight: np.ndarray) -> np.ndarray:
    from concourse.bass_utils import run_bass_kernel_spmd

    input = np.ascontiguousarray(input, dtype=np.float32)
    weight = np.ascontiguousarray(weight, dtype=np.float32)
    nc = _get()
    in_maps = []
    for i in range(NCORE):
        a, b = _swizzle_core(
            input[i * NB : (i + 1) * NB], weight[i * NB : (i + 1) * NB]
        )
        in_maps.append({"x_t": a, "w_t": b})
    res = run_bass_kernel_spmd(nc, in_maps, core_ids=list(range(NCORE)))
    return np.concatenate(
        [_unswizzle_core(res.results[i]["o_t"]) for i in range(NCORE)], axis=0
    )
